# revision 11
# baseline (speedup 1.0000x reference)
"""Trainium2 Bass kernel for AdaptiveHierarchicalPooling (segment_reduce).

Sharding: 64 graphs over 8 cores -> 8 graphs/core, fully local (weights
replicated, no collectives). All matmuls in bf16 (fp32 PSUM accumulate).

Per-graph pipeline on each core (NPG=4096 nodes, H=256, C0=256, C1=64):
  S1: logits = x @ W0            (lhsT = x^T tiles from host, rhs = W0)
  softmax: E = exp(logits); Etilde = E * exp(b0); Dw = rowsum(Etilde)
           (fused DVE tensor_tensor_reduce); A0 = Etilde / Dw
  S2: f0 = A0^T @ x              (accumulate over 32 node-tiles)
  f0T via PE transpose
  S3: logits1 = f0 @ W1 ; same softmax trick with exp(b1)
  S4: f1T = f0^T @ A1            -> gathered into F1T [128, 8g*...]
  S5: out = relu(f1_all @ Wf + bf)  (128 accumulating matmuls, M=8)
"""

import numpy as np
import ml_dtypes

import concourse.bass as bass
import concourse.tile as tile
from concourse import bacc, mybir
from concourse.bass_utils import run_bass_kernel_spmd

# Problem constants (hardcoded; kernel.py must be self-contained)
B = 64
NPG = 4096
H = 256
C0 = 256
C1 = 64
NCORES = 8
G = B // NCORES          # graphs per core = 8
NT = NPG // 128          # node tiles per graph = 32
NPC = G * NPG            # nodes per core = 32768
KT5 = (C1 * H) // 128    # final-matmul k-tiles = 128

BF = mybir.dt.bfloat16
F32 = mybir.dt.float32
EXP = mybir.ActivationFunctionType.Exp
MULT = mybir.AluOpType.mult
XOR = mybir.AluOpType.bitwise_xor
I32 = mybir.dt.int32
RC0, RC1, RC2 = -0.23549792, 2.0017324, 2.0
ADD = mybir.AluOpType.add

_NC_CACHE = None


def build_nc(reps=1):
    nc = bacc.Bacc(
        "TRN2",
        target_bir_lowering=False,
        debug=False,
        num_devices=NCORES,
    )

    xT_d = nc.dram_tensor("xT", [H, NPC], BF, kind="ExternalInput")
    # xn/wf are host-preswizzled so every DMA line is per-partition
    # contiguous (16KB / 4KB bursts) - minimal descriptor count
    xn_d = nc.dram_tensor("xn", [128, G * NT * H], BF, kind="ExternalInput")
    w0_d = nc.dram_tensor("w0", [H, C0], BF, kind="ExternalInput")
    w1_d = nc.dram_tensor("w1", [H, C1], BF, kind="ExternalInput")
    wf_d = nc.dram_tensor("wf", [128, 16 * 8 * H], BF, kind="ExternalInput")
    eb0_d = nc.dram_tensor("eb0", [128, C0], BF, kind="ExternalInput")
    eb1_d = nc.dram_tensor("eb1", [128, C1], BF, kind="ExternalInput")
    bfb_d = nc.dram_tensor("bfb", [G, H], F32, kind="ExternalInput")
    id_d = nc.dram_tensor("ident", [128, 128], BF, kind="ExternalInput")
    out_d = nc.dram_tensor("out", [G, H], F32, kind="ExternalOutput")

    with tile.TileContext(nc) as tc:
        with (
            tc.tile_pool(name="const", bufs=1) as constp,
            tc.tile_pool(name="xT", bufs=3) as xTp,
            tc.tile_pool(name="xn", bufs=3) as xnp,
            tc.tile_pool(name="e0", bufs=8) as e0p,
            tc.tile_pool(name="a0", bufs=26) as a0p,
            tc.tile_pool(name="dw", bufs=3) as dwp,
            tc.tile_pool(name="scr", bufs=3) as scrp,
            tc.tile_pool(name="f0", bufs=2) as f0p,
            tc.tile_pool(name="wf", bufs=16) as wfp,
            tc.tile_pool(name="f1t", bufs=1) as f1tp,
            tc.tile_pool(name="osb", bufs=1) as osbp,
            tc.tile_pool(name="ps_l", bufs=5, space="PSUM") as ps_l,
            tc.tile_pool(name="ps_sm", bufs=1, space="PSUM") as ps_sm,
            tc.tile_pool(name="ps_f0", bufs=1, space="PSUM") as ps_f0,
            tc.tile_pool(name="ps_t", bufs=1, space="PSUM") as ps_t,

        ):

            def recip_pool(out, in_, s1, s2):
                """1/in_ via bit-trick seed + 2 Newton passes, standard DVE
                ALU ops (avoids the fixed-cost custom-DVE ISA reciprocal).
                GpSimd rejects these opcodes (walrus ISA check), so DVE."""
                g = nc.vector
                g.tensor_scalar(s1.bitcast(I32), in_.bitcast(I32),
                                -1, None, XOR)
                g.tensor_scalar_mul(s1, s1, RC0)
                g.scalar_tensor_tensor(s2, in_, -1.0, s1, MULT, MULT)
                g.scalar_tensor_tensor(s1, s2, RC1, s1, ADD, MULT)
                g.scalar_tensor_tensor(s2, in_, -1.0, s1, MULT, MULT)
                g.scalar_tensor_tensor(out, s2, RC2, s1, ADD, MULT)

            # constants
            w0sb = constp.tile([128, 2, C0], BF)
            nc.sync.dma_start(w0sb[:, 0, :], w0_d[0:128, :])
            nc.sync.dma_start(w0sb[:, 1, :], w0_d[128:256, :])
            w1sb = constp.tile([128, 2, C1], BF)
            nc.sync.dma_start(w1sb[:, 0, :], w1_d[0:128, :])
            nc.sync.dma_start(w1sb[:, 1, :], w1_d[128:256, :])
            eb0sb = constp.tile([128, C0], BF)
            nc.sync.dma_start(eb0sb[:], eb0_d[:])
            eb1sb = constp.tile([128, C1], BF)
            nc.sync.dma_start(eb1sb[:], eb1_d[:])
            bfsb = constp.tile([G, H], F32)
            nc.sync.dma_start(bfsb[:], bfb_d[:])
            ident = constp.tile([128, 128], BF)
            nc.sync.dma_start(ident[:], id_d[:])

            # Wf resident across reps: 16 chunks of 8 k-tiles each,
            # loaded once (weights are invariant - keep them in SBUF)
            wf_view = wf_d[:].rearrange("p (c kt h) -> c p kt h",
                                        kt=8, h=H)
            wf_chunks = []
            for j in range(16):
                wft = wfp.tile([128, 8, H], BF, tag="wf", name=f"wf{j}")
                nc.sync.dma_start(wft[:], wf_view[j])
                wf_chunks.append(wft)

            for rep in range(reps):
                # F1T: [128, (hh*64+c1)*8 + g] holds f1T over all graphs
                F1T = f1tp.tile([128, 2 * C1 * G], BF)
                F1Tv = F1T[:].rearrange("p (ci g) -> p ci g", g=G)

                xn_view = xn_d[:].rearrange("p (g t h) -> g p t h",
                                            g=G, t=NT)

                pending = []

                def level1_rest(g, f0):
                        tps = ps_t.tile([128, 512], BF, tag="pst")
                        for hh in range(2):
                            for mt in range(2):
                                q = hh * 2 + mt
                                nc.tensor.transpose(
                                    tps[:, q * 128:(q + 1) * 128],
                                    f0[:, mt, hh * 128:(hh + 1) * 128],
                                    ident[:])
                        f0T = f0p.tile([128, 2, H], BF, tag="f0T")
                        nc.vector.tensor_copy(f0T[:, 0, :], tps[:, 0:256])
                        nc.vector.tensor_copy(f0T[:, 1, :], tps[:, 256:512])

                        # level 1
                        l1ps = ps_sm.tile([128, 128], F32, tag="sm")
                        for mt in range(2):
                            for hh in range(2):
                                nc.tensor.matmul(
                                    l1ps[:, mt * C1:(mt + 1) * C1],
                                    f0T[:, hh, mt * 128:(mt + 1) * 128],
                                    w1sb[:, hh, :],
                                    start=(hh == 0), stop=(hh == 1))
                        e1 = e0p.tile([128, 2, C1], BF, tag="e1")
                        a1 = a0p.tile([128, 2, C1], BF, tag="a1")
                        dw1 = dwp.tile([128, 2], F32, tag="dw1")
                        r1 = dwp.tile([128, 2], F32, tag="r1")
                        sc1 = scrp.tile([128, 2], F32, tag="sc1")
                        sc2 = scrp.tile([128, 2], F32, tag="sc2")
                        for mt in range(2):
                            nc.scalar.activation(e1[:, mt, :],
                                                 l1ps[:, mt * C1:(mt + 1) * C1], EXP)
                            nc.vector.scalar_tensor_tensor(
                                a1[:, mt, :], e1[:, mt, :], 1.0, eb1sb[:], MULT, MULT,
                                accum_out=dw1[:, mt:mt + 1])
                        recip_pool(r1[:], dw1[:], sc1[:], sc2[:])
                        for mt in range(2):
                            nc.vector.tensor_scalar_mul(a1[:, mt, :], a1[:, mt, :],
                                                        r1[:, mt:mt + 1])

                        f1ps = ps_sm.tile([128, 128], F32, tag="sm")
                        for hh in range(2):
                            for kt in range(2):
                                nc.tensor.matmul(
                                    f1ps[:, hh * C1:(hh + 1) * C1],
                                    f0[:, kt, hh * 128:(hh + 1) * 128],
                                    a1[:, kt, :],
                                    start=(kt == 0), stop=(kt == 1))
                        # scatter into F1T: col = (hh*64 + c1)*8 + g
                        for hh in range(2):
                            nc.scalar.copy(
                                F1Tv[:, hh * C1:(hh + 1) * C1, g],
                                f1ps[:, hh * C1:(hh + 1) * C1])

                for g in range(G):
                    if pending:
                        level1_rest(*pending.pop(0))

                    xT = xTp.tile([128, 2, NPG], BF, tag="xT")
                    xn = xnp.tile([128, NT, H], BF, tag="xn")
                    for kt in range(2):
                        nc.sync.dma_start(
                            xT[:, kt, :],
                            xT_d[kt * 128:(kt + 1) * 128,
                                 g * NPG:(g + 1) * NPG])
                    nc.sync.dma_start(xn[:], xn_view[g])

                    f0psb = ps_f0.tile([128, 2 * H], F32, tag="f0ps")
                    f0ps = [f0psb[:, mt * H:(mt + 1) * H] for mt in range(2)]
                    dw = dwp.tile([128, NT], F32, tag="dw")
                    r = dwp.tile([128, NT], F32, tag="r")

                    # level 0: 8 quads of 4 node-tiles. Per quad:
                    # S1 matmuls -> one exp over [128,512] -> 2 fused
                    # mul+rowsum (STT). One recip per 2 quads, then
                    # scales + S2 accumulation for both quads.
                    a0s = []
                    for q in range(NT // 4):
                        for i in range(4):
                            t = q * 4 + i
                            if i % 2 == 0:
                                lps = ps_l.tile([128, 512], F32, tag="lps")
                            ls = lps[:, (i % 2) * 256:(i % 2) * 256 + 256]
                            nc.tensor.matmul(
                                ls, xT[:, 0, t * 128:(t + 1) * 128], w0sb[:, 0, :],
                                start=True, stop=False)
                            nc.tensor.matmul(
                                ls, xT[:, 1, t * 128:(t + 1) * 128], w0sb[:, 1, :],
                                start=False, stop=True)
                            if i % 2 == 1:
                                e0 = e0p.tile([128, 512], BF, tag="e0")
                                nc.scalar.activation(e0[:], lps[:], EXP)
                                for k in range(2):
                                    tk = t - 1 + k
                                    a0 = a0p.tile([128, C0], BF, tag="a0",
                                                  name=f"a0_{tk}")
                                    nc.vector.scalar_tensor_tensor(
                                        a0[:], e0[:, k * C0:(k + 1) * C0],
                                        1.0, eb0sb[:], MULT, MULT,
                                        accum_out=dw[:, tk:tk + 1])
                                    a0s.append(a0)
                        if q % 2 == 1:
                            t0 = q * 4 - 4
                            sg1 = scrp.tile([128, 8], F32, tag="sg1")
                            sg2 = scrp.tile([128, 8], F32, tag="sg2")
                            recip_pool(r[:, t0:t0 + 8], dw[:, t0:t0 + 8],
                                       sg1[:], sg2[:])
                            for k in range(8):
                                t = t0 + k
                                a0 = a0s[t]
                                # all scales on DVE (4x mode there; ACT is
                                # exp-bound)
                                nc.vector.tensor_scalar_mul(
                                    a0[:], a0[:], r[:, t:t + 1])
                                for mt in range(2):
                                    # one start for the whole bank: start=True
                                    # clears has_written for ALL bank elements,
                                    # so a second start (mt=1) would wipe the
                                    # mt=0 chain's accumulation flags
                                    nc.tensor.matmul(
                                        f0ps[mt],
                                        a0[:, mt * 128:(mt + 1) * 128],
                                        xn[:, t, :],
                                        start=(t == 0 and mt == 0),
                                        stop=(t == NT - 1))

                    # evacuate f0 (c0-part, h) and build f0T (h-part, c0)
                    f0 = f0p.tile([128, 2, H], BF, tag="f0")
                    nc.scalar.copy(f0[:, 0, :], f0ps[0])
                    nc.scalar.copy(f0[:, 1, :], f0ps[1])

                    pending.append((g, f0))

                while pending:
                    level1_rest(*pending.pop(0))

                # S5: out = relu(f1_all @ Wf + bf). 4-way col-tiled: k-tiles
                # split over 4 PE column groups, partials at partition 32j of
                # 4 separate PSUM banks, combined via DMA gather + DVE adds.
                # Issue order interleaves the 4 col-groups (j = kt % 4) so
                # consecutive matmuls land on disjoint PE column strips and
                # run concurrently (m=8 uses 8/128 cols; serial issue within
                # one group would leave the array 94% idle).
                s5ps = []
                for j, pool, ptag in ((0, ps_l, "lps"), (1, ps_l, "lps"),
                                      (2, ps_f0, "f0ps"), (3, ps_t, "pst")):
                    s5p = pool.tile([128, H], F32, tag=ptag,
                                    name=f"s5ps{j}")
                    s5ps.append(s5p)
                for kt in range(KT5):
                    c1, hh = kt // 2, kt % 2
                    ci = hh * C1 + c1
                    j = kt % 4
                    nc.tensor.matmul(
                        s5ps[j][32 * j:32 * j + G, :],
                        F1Tv[:, ci, :],
                        wf_chunks[kt // 8][:, kt % 8, :],
                        start=(kt < 4), stop=(kt >= KT5 - 4),
                        tile_position=(0, 32 * j))
                s5e = osbp.tile([128, H], F32, tag="s5e")
                for j in range(4):
                    nc.vector.tensor_copy(s5e[32 * j:32 * j + G, :],
                                          s5ps[j][32 * j:32 * j + G, :])
                s5sb = osbp.tile([G, 4, H], F32, tag="s5sb")
                for j in range(4):
                    nc.sync.dma_start(s5sb[:, j, :],
                                      s5e[32 * j:32 * j + G, :])
                osb = osbp.tile([G, H], F32)
                nc.vector.tensor_add(osb[:], s5sb[:, 0, :], s5sb[:, 1, :])
                nc.vector.tensor_add(osb[:], osb[:], s5sb[:, 2, :])
                nc.vector.tensor_add(osb[:], osb[:], s5sb[:, 3, :])
                nc.vector.tensor_add(osb[:], osb[:], bfsb[:])
                nc.vector.tensor_scalar_max(osb[:], osb[:], 0.0)
                nc.sync.dma_start(out_d[:], osb[:])

    nc.compile()
    return nc


def _get_nc():
    global _NC_CACHE
    if _NC_CACHE is None:
        _NC_CACHE = build_nc()
    return _NC_CACHE


def _make_in_maps(x, W0, b0, W1, b1, Wf, bf):
    bf16 = ml_dtypes.bfloat16
    w0b = np.ascontiguousarray(W0.astype(bf16))
    w1b = np.ascontiguousarray(W1.astype(bf16))
    # wf swizzled: wf_s[p, c, kt, h] = Wf[(c*8+kt)*128+p, h], flattened to
    # [128, 16*8*H] so each chunk DMA is 4KB-contiguous per partition
    wfb = np.ascontiguousarray(
        Wf.astype(bf16).reshape(16, 8, 128, H)
        .transpose(2, 0, 1, 3).reshape(128, -1))
    eb0 = np.broadcast_to(np.exp(b0.astype(np.float64)).astype(bf16)[None, :],
                          (128, C0)).copy()
    eb1 = np.broadcast_to(np.exp(b1.astype(np.float64)).astype(bf16)[None, :],
                          (128, C1)).copy()
    bfb = np.broadcast_to(bf.astype(np.float32)[None, :], (G, H)).copy()
    ident = np.eye(128, dtype=bf16)

    in_maps = []
    for c in range(NCORES):
        xs = x[c * NPC:(c + 1) * NPC]
        # xn swizzled: xn_s[p, g, t, h] = x[g*NPG + t*128 + p, h], flattened
        # to [128, G*NT*H] so each graph's DMA is 16KB-contiguous/partition
        xnb = np.ascontiguousarray(
            xs.astype(bf16).reshape(G, NT, 128, H)
            .transpose(2, 0, 1, 3).reshape(128, -1))
        xTb = np.ascontiguousarray(xs.T.astype(bf16))
        in_maps.append({
            "xT": xTb, "xn": xnb, "w0": w0b, "w1": w1b, "wf": wfb,
            "eb0": eb0, "eb1": eb1, "bfb": bfb, "ident": ident,
        })
    return in_maps


def run(x, W0, b0, W1, b1, Wf, bf, trace=False):
    nc = _get_nc()
    in_maps = _make_in_maps(x, W0, b0, W1, b1, Wf, bf)
    res = run_bass_kernel_spmd(nc, in_maps, core_ids=list(range(NCORES)),
                               trace=trace)
    out = np.concatenate([np.asarray(res.results[c]["out"], dtype=np.float32)
                          for c in range(NCORES)], axis=0)
    return out, res


def kernel(x, edge_index, batch, W0, b0, W1, b1, Wf, bf):
    x = np.asarray(x, dtype=np.float32)
    out, _ = run(np.asarray(x, np.float32), np.asarray(W0, np.float32),
                 np.asarray(b0, np.float32), np.asarray(W1, np.float32),
                 np.asarray(b1, np.float32), np.asarray(Wf, np.float32),
                 np.asarray(bf, np.float32))
    return out



# revision 12
# speedup vs baseline: 1.2325x; 1.2325x over previous
"""Trainium2 Bass kernel for AdaptiveHierarchicalPooling (segment_reduce).

Sharding: 64 graphs over 8 cores -> 8 graphs/core, fully local (weights
replicated, no collectives). All matmuls in bf16 (fp32 PSUM accumulate).

Per-graph pipeline on each core (NPG=4096 nodes, H=256, C0=256, C1=64):
  S1: logits = x @ W0            (lhsT = x^T tiles from host, rhs = W0)
  softmax: E = exp(logits); Etilde = E * exp(b0); Dw = rowsum(Etilde)
           (fused DVE tensor_tensor_reduce); A0 = Etilde / Dw
  S2: f0 = A0^T @ x              (accumulate over 32 node-tiles)
  f0T via PE transpose
  S3: logits1 = f0 @ W1 ; same softmax trick with exp(b1)
  S4: f1T = f0^T @ A1            -> gathered into F1T [128, 8g*...]
  S5: out = relu(f1_all @ Wf + bf)  (128 accumulating matmuls, M=8)
"""

import numpy as np
import ml_dtypes

import concourse.bass as bass
import concourse.tile as tile
from concourse import bacc, mybir
from concourse.bass_utils import run_bass_kernel_spmd

# Problem constants (hardcoded; kernel.py must be self-contained)
B = 64
NPG = 4096
H = 256
C0 = 256
C1 = 64
NCORES = 8
G = B // NCORES          # graphs per core = 8
NT = NPG // 128          # node tiles per graph = 32
NPC = G * NPG            # nodes per core = 32768
KT5 = (C1 * H) // 128    # final-matmul k-tiles = 128

BF = mybir.dt.bfloat16
F32 = mybir.dt.float32
EXP = mybir.ActivationFunctionType.Exp
MULT = mybir.AluOpType.mult
XOR = mybir.AluOpType.bitwise_xor
I32 = mybir.dt.int32
RC0, RC1, RC2 = -0.23549792, 2.0017324, 2.0
ADD = mybir.AluOpType.add

_NC_CACHE = None


def build_nc(reps=1):
    nc = bacc.Bacc(
        "TRN2",
        target_bir_lowering=False,
        debug=False,
        num_devices=NCORES,
    )

    xT_d = nc.dram_tensor("xT", [H, NPC], BF, kind="ExternalInput")
    # xn/wf are host-preswizzled so every DMA line is per-partition
    # contiguous (16KB / 4KB bursts) - minimal descriptor count
    xn_d = nc.dram_tensor("xn", [128, G * NT * H], BF, kind="ExternalInput")
    w0_d = nc.dram_tensor("w0", [H, C0], BF, kind="ExternalInput")
    w1_d = nc.dram_tensor("w1", [H, C1], BF, kind="ExternalInput")
    wf_d = nc.dram_tensor("wf", [128, 16 * 8 * H], BF, kind="ExternalInput")
    eb0_d = nc.dram_tensor("eb0", [128, C0], BF, kind="ExternalInput")
    eb1_d = nc.dram_tensor("eb1", [128, C1], BF, kind="ExternalInput")
    bfb_d = nc.dram_tensor("bfb", [G, H], F32, kind="ExternalInput")
    id_d = nc.dram_tensor("ident", [128, 128], BF, kind="ExternalInput")
    out_d = nc.dram_tensor("out", [G, H], F32, kind="ExternalOutput")

    with tile.TileContext(nc) as tc:
        with (
            tc.tile_pool(name="const", bufs=1) as constp,
            tc.tile_pool(name="xT", bufs=3) as xTp,
            tc.tile_pool(name="xn", bufs=3) as xnp,
            tc.tile_pool(name="e0", bufs=8) as e0p,
            tc.tile_pool(name="a0", bufs=26) as a0p,
            tc.tile_pool(name="dw", bufs=3) as dwp,
            tc.tile_pool(name="scr", bufs=3) as scrp,
            tc.tile_pool(name="f0", bufs=2) as f0p,
            tc.tile_pool(name="wf", bufs=16) as wfp,
            tc.tile_pool(name="f1t", bufs=1) as f1tp,
            tc.tile_pool(name="osb", bufs=1) as osbp,
            tc.tile_pool(name="ps_l", bufs=5, space="PSUM") as ps_l,
            tc.tile_pool(name="ps_sm", bufs=1, space="PSUM") as ps_sm,
            tc.tile_pool(name="ps_f0", bufs=1, space="PSUM") as ps_f0,
            tc.tile_pool(name="ps_t", bufs=1, space="PSUM") as ps_t,

        ):

            def recip_pool(out, in_, s1, s2):
                """1/in_ via bit-trick seed + 2 Newton passes, standard DVE
                ALU ops (avoids the fixed-cost custom-DVE ISA reciprocal).
                GpSimd rejects these opcodes (walrus ISA check), so DVE."""
                g = nc.vector
                g.tensor_scalar(s1.bitcast(I32), in_.bitcast(I32),
                                -1, None, XOR)
                g.tensor_scalar_mul(s1, s1, RC0)
                g.scalar_tensor_tensor(s2, in_, -1.0, s1, MULT, MULT)
                g.scalar_tensor_tensor(s1, s2, RC1, s1, ADD, MULT)
                g.scalar_tensor_tensor(s2, in_, -1.0, s1, MULT, MULT)
                g.scalar_tensor_tensor(out, s2, RC2, s1, ADD, MULT)

            # constants
            w0sb = constp.tile([128, 2, C0], BF)
            nc.sync.dma_start(w0sb[:, 0, :], w0_d[0:128, :])
            nc.sync.dma_start(w0sb[:, 1, :], w0_d[128:256, :])
            w1sb = constp.tile([128, 2, C1], BF)
            nc.sync.dma_start(w1sb[:, 0, :], w1_d[0:128, :])
            nc.sync.dma_start(w1sb[:, 1, :], w1_d[128:256, :])
            eb0sb = constp.tile([128, C0], BF)
            nc.sync.dma_start(eb0sb[:], eb0_d[:])
            eb1sb = constp.tile([128, C1], BF)
            nc.sync.dma_start(eb1sb[:], eb1_d[:])
            bfsb = constp.tile([G, H], F32)
            nc.sync.dma_start(bfsb[:], bfb_d[:])
            ident = constp.tile([128, 128], BF)
            nc.sync.dma_start(ident[:], id_d[:])

            # Wf resident across reps: 16 chunks of 8 k-tiles each,
            # loaded once (weights are invariant - keep them in SBUF)
            wf_view = wf_d[:].rearrange("p (c kt h) -> c p kt h",
                                        kt=8, h=H)
            wf_chunks = []
            for j in range(16):
                wft = wfp.tile([128, 8, H], BF, tag="wf", name=f"wf{j}")
                nc.sync.dma_start(wft[:], wf_view[j])
                wf_chunks.append(wft)

            for rep in range(reps):
                # F1T: [128, (hh*64+c1)*8 + g] holds f1T over all graphs
                F1T = f1tp.tile([128, 2 * C1 * G], BF)
                F1Tv = F1T[:].rearrange("p (ci g) -> p ci g", g=G)

                xn_view = xn_d[:].rearrange("p (g t h) -> g p t h",
                                            g=G, t=NT)

                pending = []

                def level1_rest(g, f0):
                        tps = ps_t.tile([128, 512], BF, tag="pst")
                        for hh in range(2):
                            for mt in range(2):
                                q = hh * 2 + mt
                                nc.tensor.transpose(
                                    tps[:, q * 128:(q + 1) * 128],
                                    f0[:, mt, hh * 128:(hh + 1) * 128],
                                    ident[:])
                        f0T = f0p.tile([128, 2, H], BF, tag="f0T")
                        nc.scalar.copy(f0T[:, 0, :], tps[:, 0:256])
                        nc.scalar.copy(f0T[:, 1, :], tps[:, 256:512])

                        # level 1
                        l1ps = ps_sm.tile([128, 128], F32, tag="sm")
                        for mt in range(2):
                            for hh in range(2):
                                nc.tensor.matmul(
                                    l1ps[:, mt * C1:(mt + 1) * C1],
                                    f0T[:, hh, mt * 128:(mt + 1) * 128],
                                    w1sb[:, hh, :],
                                    start=(hh == 0), stop=(hh == 1))
                        e1 = e0p.tile([128, 2, C1], BF, tag="e1")
                        a1 = a0p.tile([128, 2, C1], BF, tag="a1")
                        dw1 = dwp.tile([128, 2], F32, tag="dw1")
                        r1 = dwp.tile([128, 2], F32, tag="r1")
                        sc1 = scrp.tile([128, 2], F32, tag="sc1")
                        sc2 = scrp.tile([128, 2], F32, tag="sc2")
                        for mt in range(2):
                            nc.scalar.activation(e1[:, mt, :],
                                                 l1ps[:, mt * C1:(mt + 1) * C1], EXP)
                            nc.vector.scalar_tensor_tensor(
                                a1[:, mt, :], e1[:, mt, :], 1.0, eb1sb[:], MULT, MULT,
                                accum_out=dw1[:, mt:mt + 1])
                        recip_pool(r1[:], dw1[:], sc1[:], sc2[:])
                        for mt in range(2):
                            nc.vector.tensor_scalar_mul(a1[:, mt, :], a1[:, mt, :],
                                                        r1[:, mt:mt + 1])

                        f1ps = ps_sm.tile([128, 128], F32, tag="sm")
                        for hh in range(2):
                            for kt in range(2):
                                nc.tensor.matmul(
                                    f1ps[:, hh * C1:(hh + 1) * C1],
                                    f0[:, kt, hh * 128:(hh + 1) * 128],
                                    a1[:, kt, :],
                                    start=(kt == 0), stop=(kt == 1))
                        # scatter into F1T: col = (hh*64 + c1)*8 + g
                        for hh in range(2):
                            nc.scalar.copy(
                                F1Tv[:, hh * C1:(hh + 1) * C1, g],
                                f1ps[:, hh * C1:(hh + 1) * C1])

                for g in range(G):
                    if pending:
                        level1_rest(*pending.pop(0))

                    xT = xTp.tile([128, 2, NPG], BF, tag="xT")
                    xn = xnp.tile([128, NT, H], BF, tag="xn")
                    for kt in range(2):
                        nc.sync.dma_start(
                            xT[:, kt, :],
                            xT_d[kt * 128:(kt + 1) * 128,
                                 g * NPG:(g + 1) * NPG])
                    nc.sync.dma_start(xn[:], xn_view[g])

                    f0psb = ps_f0.tile([128, 2 * H], F32, tag="f0ps")
                    f0ps = [f0psb[:, mt * H:(mt + 1) * H] for mt in range(2)]
                    dw = dwp.tile([128, NT], F32, tag="dw")
                    r = dwp.tile([128, NT], F32, tag="r")

                    # level 0: 8 quads of 4 node-tiles. Per quad:
                    # S1 matmuls -> one exp over [128,512] -> 2 fused
                    # mul+rowsum (STT). One recip per 2 quads, then
                    # scales + S2 accumulation for both quads.
                    a0s = []
                    for q in range(NT // 4):
                        for i in range(4):
                            t = q * 4 + i
                            if i % 2 == 0:
                                lps = ps_l.tile([128, 512], F32, tag="lps")
                            ls = lps[:, (i % 2) * 256:(i % 2) * 256 + 256]
                            nc.tensor.matmul(
                                ls, xT[:, 0, t * 128:(t + 1) * 128], w0sb[:, 0, :],
                                start=True, stop=False)
                            nc.tensor.matmul(
                                ls, xT[:, 1, t * 128:(t + 1) * 128], w0sb[:, 1, :],
                                start=False, stop=True)
                            if i % 2 == 1:
                                e0 = e0p.tile([128, 512], BF, tag="e0")
                                nc.scalar.activation(e0[:], lps[:], EXP)
                                for k in range(2):
                                    tk = t - 1 + k
                                    a0 = a0p.tile([128, C0], BF, tag="a0",
                                                  name=f"a0_{tk}")
                                    nc.vector.scalar_tensor_tensor(
                                        a0[:], e0[:, k * C0:(k + 1) * C0],
                                        1.0, eb0sb[:], MULT, MULT,
                                        accum_out=dw[:, tk:tk + 1])
                                    a0s.append(a0)
                        if q % 2 == 1:
                            t0 = q * 4 - 4
                            sg1 = scrp.tile([128, 8], F32, tag="sg1")
                            sg2 = scrp.tile([128, 8], F32, tag="sg2")
                            recip_pool(r[:, t0:t0 + 8], dw[:, t0:t0 + 8],
                                       sg1[:], sg2[:])
                            for k in range(8):
                                t = t0 + k
                                a0 = a0s[t]
                                if k % 4 == 1:
                                    # 2/8 scales on Act to unload DVE
                                    nc.scalar.mul(a0[:], a0[:], r[:, t:t + 1])
                                else:
                                    nc.vector.tensor_scalar_mul(
                                        a0[:], a0[:], r[:, t:t + 1])
                                for mt in range(2):
                                    # one start for the whole bank: start=True
                                    # clears has_written for ALL bank elements,
                                    # so a second start (mt=1) would wipe the
                                    # mt=0 chain's accumulation flags
                                    nc.tensor.matmul(
                                        f0ps[mt],
                                        a0[:, mt * 128:(mt + 1) * 128],
                                        xn[:, t, :],
                                        start=(t == 0 and mt == 0),
                                        stop=(t == NT - 1))

                    # evacuate f0 (c0-part, h) and build f0T (h-part, c0)
                    f0 = f0p.tile([128, 2, H], BF, tag="f0")
                    nc.scalar.copy(f0[:, 0, :], f0ps[0])
                    nc.scalar.copy(f0[:, 1, :], f0ps[1])

                    pending.append((g, f0))

                while pending:
                    level1_rest(*pending.pop(0))

                # S5: out = relu(f1_all @ Wf + bf). 4-way col-tiled: k-tiles
                # split over 4 PE column groups, partials at partition 32j of
                # 4 separate PSUM banks, combined via DMA gather + DVE adds.
                # Issue order interleaves the 4 col-groups (j = kt % 4) so
                # consecutive matmuls land on disjoint PE column strips and
                # run concurrently (m=8 uses 8/128 cols; serial issue within
                # one group would leave the array 94% idle).
                s5ps = []
                for j, pool, ptag in ((0, ps_l, "lps"), (1, ps_l, "lps"),
                                      (2, ps_f0, "f0ps"), (3, ps_t, "pst")):
                    s5p = pool.tile([128, H], F32, tag=ptag,
                                    name=f"s5ps{j}")
                    s5ps.append(s5p)
                for kt in range(KT5):
                    c1, hh = kt // 2, kt % 2
                    ci = hh * C1 + c1
                    j = kt % 4
                    nc.tensor.matmul(
                        s5ps[j][32 * j:32 * j + G, :],
                        F1Tv[:, ci, :],
                        wf_chunks[kt // 8][:, kt % 8, :],
                        start=(kt < 4), stop=(kt >= KT5 - 4),
                        tile_position=(0, 32 * j))
                s5e = osbp.tile([128, H], F32, tag="s5e")
                for j in range(4):
                    nc.vector.tensor_copy(s5e[32 * j:32 * j + G, :],
                                          s5ps[j][32 * j:32 * j + G, :])
                s5sb = osbp.tile([G, 4, H], F32, tag="s5sb")
                for j in range(4):
                    nc.sync.dma_start(s5sb[:, j, :],
                                      s5e[32 * j:32 * j + G, :])
                osb = osbp.tile([G, H], F32)
                nc.vector.tensor_add(osb[:], s5sb[:, 0, :], s5sb[:, 1, :])
                nc.vector.tensor_add(osb[:], osb[:], s5sb[:, 2, :])
                nc.vector.tensor_add(osb[:], osb[:], s5sb[:, 3, :])
                nc.vector.tensor_add(osb[:], osb[:], bfsb[:])
                nc.vector.tensor_scalar_max(osb[:], osb[:], 0.0)
                nc.sync.dma_start(out_d[:], osb[:])

    nc.compile()
    return nc


def _get_nc():
    global _NC_CACHE
    if _NC_CACHE is None:
        _NC_CACHE = build_nc()
    return _NC_CACHE


def _make_in_maps(x, W0, b0, W1, b1, Wf, bf):
    bf16 = ml_dtypes.bfloat16
    w0b = np.ascontiguousarray(W0.astype(bf16))
    w1b = np.ascontiguousarray(W1.astype(bf16))
    # wf swizzled: wf_s[p, c, kt, h] = Wf[(c*8+kt)*128+p, h], flattened to
    # [128, 16*8*H] so each chunk DMA is 4KB-contiguous per partition
    wfb = np.ascontiguousarray(
        Wf.astype(bf16).reshape(16, 8, 128, H)
        .transpose(2, 0, 1, 3).reshape(128, -1))
    eb0 = np.broadcast_to(np.exp(b0.astype(np.float64)).astype(bf16)[None, :],
                          (128, C0)).copy()
    eb1 = np.broadcast_to(np.exp(b1.astype(np.float64)).astype(bf16)[None, :],
                          (128, C1)).copy()
    bfb = np.broadcast_to(bf.astype(np.float32)[None, :], (G, H)).copy()
    ident = np.eye(128, dtype=bf16)

    in_maps = []
    for c in range(NCORES):
        xs = x[c * NPC:(c + 1) * NPC]
        # xn swizzled: xn_s[p, g, t, h] = x[g*NPG + t*128 + p, h], flattened
        # to [128, G*NT*H] so each graph's DMA is 16KB-contiguous/partition
        xnb = np.ascontiguousarray(
            xs.astype(bf16).reshape(G, NT, 128, H)
            .transpose(2, 0, 1, 3).reshape(128, -1))
        xTb = np.ascontiguousarray(xs.T.astype(bf16))
        in_maps.append({
            "xT": xTb, "xn": xnb, "w0": w0b, "w1": w1b, "wf": wfb,
            "eb0": eb0, "eb1": eb1, "bfb": bfb, "ident": ident,
        })
    return in_maps


def run(x, W0, b0, W1, b1, Wf, bf, trace=False):
    nc = _get_nc()
    in_maps = _make_in_maps(x, W0, b0, W1, b1, Wf, bf)
    res = run_bass_kernel_spmd(nc, in_maps, core_ids=list(range(NCORES)),
                               trace=trace)
    out = np.concatenate([np.asarray(res.results[c]["out"], dtype=np.float32)
                          for c in range(NCORES)], axis=0)
    return out, res


def kernel(x, edge_index, batch, W0, b0, W1, b1, Wf, bf):
    x = np.asarray(x, dtype=np.float32)
    out, _ = run(np.asarray(x, np.float32), np.asarray(W0, np.float32),
                 np.asarray(b0, np.float32), np.asarray(W1, np.float32),
                 np.asarray(b1, np.float32), np.asarray(Wf, np.float32),
                 np.asarray(bf, np.float32))
    return out

